# revision 59
# baseline (speedup 1.0000x reference)
"""Trainium2 Bass kernel for nn_Distance (exact EDT + Gaussian click maps).

Computes, for inputs [4, 320, 320, 2] f32 in [0,1):
  restored = uint8((1-x)*127.5); zero-mask = (restored == 0)
  d2 = squared Euclidean distance transform of the zero-mask
       (per image, channel folded into batch -> 8 independent images)
  out[..., c*3+s] = exp(-d2_c / (2*sigma_s^2)), sigmas = [0.02,0.08,0.16]*320

Sharding: pure data parallel, one folded image (b, c) per NeuronCore (8 cores).

Device algorithm (decomposed transposed relative to the reference):
  phase A: per-row 1D distances along W via two fused distance-recurrence
           scans on DVE (state = nm*state + nm gives 0 at seeds, +1 runs);
           the zero-mask is a single threshold compare x < T (T found by
           exact fp32 bisection; provably equal to the uint8-trunc mask).
  phase B: d2[j,i] = min_{|k|<=R} g2t[j,i+k] + k^2, split into
    - near band |k| < KF=5: exact windowed pair-min + k^2-add + log-fold
      on DVE over PE-transposed fp16 row-distance squares, processed per
      160-col i-half so merges/exps/stores pipeline into ACT/SP early
      (Pool has no tensor-min; it runs masks, E-squares and the wc2
      repack instead);
    - far band KF <= |k| <= R: softmin on the TensorEngine:
      M[j,i] = sum_y exp(-s*g2[y,j]) * exp(-s*((y-i)^2-KF^2)) accumulated
      over y-chunks in PSUM (bf16 matmuls vs a constant banded matrix;
      bf16 keeps fp32's exponent range), then d2far = KF^2 - ln(M)/s
      (ACT Ln with an SBUF eps bias + DVE affine). s is chosen on the
      host so every winner-relevant term stays inside fp32 range; the
      softmin bias only touches pixels whose winner is >= KF away, where
      all three Gaussians are flat (measured end-to-end rel err ~2e-3 on
      device vs the 2e-2 gate). The activation-function table is pinned
      to natural_log_exp_and_others (square+exp+ln+copy, 400-bucket
      tables) so only one LoadActFuncSet is ever issued.
  merge + 3 exps per half-block on ACT, fused 3-plane output stores.
R and s derive from a host-side exact EDT of the actual input. The near
band is exact in fp16 (winning candidates are integers <= 2047; padded or
overflowed losers saturate to +inf and never win).
"""

import math
import os
import sys

import numpy as np

for _p in ("/opt/trn_rl_repo", "/root/.axon_site/_ro/trn_rl_repo"):
    if os.path.isdir(_p) and _p not in sys.path:
        sys.path.insert(0, _p)

import concourse.bass as bass  # noqa: E402
import concourse.tile as tile  # noqa: E402
from concourse import bacc, mybir  # noqa: E402
from concourse.ap import AP  # noqa: E402
from concourse.bass_utils import run_bass_kernel_spmd  # noqa: E402

H = 320
W = 320
NCORES = 8
BIG = 1e5
LENGTH = 320
# exact fp32 threshold: (uint8-trunc zero mask) == (x >= T_ZERO); bisected
# over fp32 and verified exhaustively around the boundary.
T_ZERO = float(np.float32(0.99215686))
C0 = 1024.0  # scan init: "no seed yet" distance offset (< 2048 for fp16)
KF_NEAR = 5  # near band: exact k in [0, KF); far band via PE softmin
DEBUG = False  # adds a dbg output with pre-exp d2 per block
LN_EPS = 1e-37  # Ln bias floor: maps empty far fields to d2 > maxd2

F32 = mybir.dt.float32
F16 = mybir.dt.float16
BF16 = mybir.dt.bfloat16
Alu = mybir.AluOpType
ActFn = mybir.ActivationFunctionType

CHUNKS = [(0, 128), (128, 128), (256, 64)]

_prog_cache: dict = {}


def _denoms():
    sig = (np.float32(np.array([0.02, 0.08, 0.16], np.float32)) * np.float32(LENGTH)).astype(np.float32)
    return (np.float32(2.0) * sig * sig).astype(np.float32)


def _win(apo, col0, ni, istep, nk, kstep):
    """3D overlapping-window view of a 2D [P, F] AP: [p, i, k] -> col0 + i*istep + k*kstep."""
    return AP(apo.tensor, apo.offset + col0, [list(apo.ap[0]), [istep, ni], [kstep, nk]])


def _build(R, s, fp16):
    """Build + compile the per-core program. Returns the Bacc module."""
    dt = F16 if fp16 else F32
    PADV = 60000.0 if fp16 else 1e20
    use_far = fp16
    KF = KF_NEAR if use_far else R + 1
    KD = KF - 1  # DVE near k in [1, 1+KD); k=0 merged separately
    assert KD >= 2
    RP = 16 if use_far else ((R + 15) // 16) * 16
    PADH = RP + H + RP
    HH = H // 2
    pack_wc2 = True
    dens = _denoms()

    nc = bacc.Bacc("TRN2", target_bir_lowering=False, debug=False, num_devices=NCORES)
    x_d = nc.dram_tensor("x", [H, W], F32, kind="ExternalInput").ap()
    NO2 = KF - 1  # o2 table: k^2 for k = 1..KF-1
    cst_d = nc.dram_tensor("consts", [128, NO2 + 128], dt, kind="ExternalInput").ap()
    wb_d = None
    if use_far:
        wb_d = nc.dram_tensor("wband", [128, 3 * W + 1], BF16, kind="ExternalInput").ap()
    y_d = nc.dram_tensor("y", [3, W, H], F32, kind="ExternalOutput").ap()
    dbg_d = None
    if DEBUG:
        dbg_d = nc.dram_tensor("dbg", [3 * 128, W], F16, kind="ExternalOutput").ap()

    with tile.TileContext(nc) as tc:
        with (
            tc.tile_pool(name="const", bufs=1) as constp,
            tc.tile_pool(name="xp", bufs=1) as xp,
            tc.tile_pool(name="pa", bufs=3) as pa,
            tc.tile_pool(name="ep", bufs=1) as ep,
            tc.tile_pool(name="g2tp", bufs=4) as g2tp,
            tc.tile_pool(name="sbig", bufs=2) as sbig,
            tc.tile_pool(name="accp", bufs=2) as accp,
            tc.tile_pool(name="d2p", bufs=3) as d2p,
            tc.tile_pool(name="outp", bufs=3) as outp,
            tc.tile_pool(name="psum", bufs=4, space="PSUM") as psump,
            tc.tile_pool(name="psmm", bufs=1, space="PSUM") as psmm,
        ):
            # ---- input DMAs (x spread over SP and ACT queues) ----
            x0 = xp.tile([128, W], F32, tag="x0")
            nc.sync.dma_start(x0[:], x_d[0:128, :])
            x1 = xp.tile([128, W], F32, tag="x1")
            nc.scalar.dma_start(x1[:], x_d[128:256, :])
            x2 = xp.tile([128, W], F32, tag="x2")
            nc.sync.dma_start(x2[:64], x_d[256:320, :])
            cst = constp.tile([128, NO2 + 128], dt)
            nc.scalar.dma_start(cst[:], cst_d)
            o2 = cst[:, 0:NO2]
            idt = cst[:, NO2 : NO2 + 128]
            wb = None
            if use_far:
                wb = constp.tile([128, 3 * W + 1], BF16, tag="wb")
                nc.scalar.dma_start(wb[:], wb_d)
            xtiles = [x0, x1, x2]

            # transposed g2 tiles (pads memset by DVE during the DMA wait)
            g2t = []
            for wc in range(3):
                t = g2tp.tile([128, PADH], dt, tag="g2t")
                nc.vector.memset(t[:, 0:RP], PADV)
                nc.vector.memset(t[:, RP + H : PADH], PADV)
                g2t.append(t)
            pk = None
            if pack_wc2:
                pk = g2tp.tile([128, RP + HH + RP], dt, tag="pk")

            # ---- phase A ----
            def phase_a_mask(hc, eng):
                h0, hs = CHUNKS[hc]
                nm = pa.tile([128, W], dt, tag="nm")
                eng.tensor_scalar(
                    nm[:hs], xtiles[hc][:hs], 1.0, T_ZERO, Alu.mult, Alu.is_lt
                )
                return nm

            def phase_a_scan(hc, nm):
                """g = min(left-dist, right-dist); no clamp -- squares of
                no-seed runs overflow fp16 to +inf, which loses every min."""
                h0, hs = CHUNKS[hc]
                dl = pa.tile([128, W], dt, tag="dl")
                nc.vector.tensor_tensor_scan(
                    dl[:hs], nm[:hs], nm[:hs], C0, Alu.mult, Alu.add
                )
                dr = pa.tile([128, W], dt, tag="dr")
                nc.vector.tensor_tensor_scan(
                    dr[:hs, ::-1], nm[:hs, ::-1], nm[:hs, ::-1], C0, Alu.mult, Alu.add
                )
                g_t = pa.tile([128, W], dt, tag="g")
                nc.vector.tensor_tensor(g_t[:hs], dl[:hs], dr[:hs], Alu.min)
                return g_t

            g2pre01 = None
            E01 = None
            E2 = None
            if use_far:
                g2pre01 = ep.tile([128, 2 * W], F32, tag="g2pre01")
                E01 = ep.tile([128, 2 * W], BF16, tag="E01")
                E2 = ep.tile([128, W], BF16, tag="E2")

            def eslice(yc, j0, jn, hs):
                if yc < 2:
                    return E01[:hs, yc * W + j0 : yc * W + j0 + jn]
                return E2[:hs, j0 : j0 + jn]

            def phase_a_post(hc, g_t):
                """PE-transpose g into PSUM, square during the ACT copy-out;
                also build the far-field factor E = exp(-s * g^2)."""
                h0, hs = CHUNKS[hc]
                for wc, (w0, ws) in enumerate(CHUNKS):
                    pt = psump.tile([128, 128], dt, tag="pt")
                    nc.tensor.transpose(
                        pt[:ws, :hs], g_t[:hs, w0 : w0 + ws], idt[:hs, :hs]
                    )
                    nc.scalar.activation(
                        g2t[wc][:ws, RP + h0 : RP + h0 + hs], pt[:ws, :hs], ActFn.Square
                    )
                if use_far:
                    if hc < 2:
                        nc.gpsimd.tensor_tensor(
                            g2pre01[:hs, hc * W : (hc + 1) * W], g_t[:hs], g_t[:hs],
                            Alu.mult,
                        )
                        if hc == 1:
                            nc.scalar.activation(
                                E01[:], g2pre01[:], ActFn.Exp, scale=float(-s)
                            )
                    else:
                        g2pre = pa.tile([128, W], F32, tag="g2pre")
                        nc.gpsimd.tensor_tensor(g2pre[:hs], g_t[:hs], g_t[:hs], Alu.mult)
                        nc.scalar.activation(
                            E2[:hs], g2pre[:hs], ActFn.Exp, scale=float(-s)
                        )

            nm0 = phase_a_mask(0, nc.vector)
            nm1 = phase_a_mask(1, nc.gpsimd)
            nm2 = phase_a_mask(2, nc.gpsimd)
            g0 = phase_a_scan(0, nm0)
            phase_a_post(0, g0)
            g1 = phase_a_scan(1, nm1)
            phase_a_post(1, g1)
            g2_ = phase_a_scan(2, nm2)
            phase_a_post(2, g2_)

            if pack_wc2:
                nc.gpsimd.tensor_copy(pk[0:64, :], g2t[2][0:64, 0 : 2 * RP + HH])
                nc.gpsimd.tensor_copy(pk[64:128, :], g2t[2][0:64, HH : 2 * RP + 2 * HH])

            # ---- phase B ----
            def near_dve(gta, col0, n_i, d2v, kd=KD):
                """d2v = min_{k in [0, 1+kd)} min(g[i+k], g[i-k]) + k^2."""
                np_ = gta.shape[0]
                S = sbig.tile([128, W * KD], dt, tag="Sd")
                assert n_i * kd <= W * KD
                S3 = AP(
                    S.tensor, S.offset,
                    [list(S[:np_].ap[0]), [kd, n_i], [1, kd]],
                )
                nc.vector.tensor_tensor(
                    S3, _win(gta, col0 + 1, n_i, 1, kd, 1),
                    _win(gta, col0 - 1, n_i, 1, kd, -1), Alu.min,
                )
                nc.vector.tensor_tensor(
                    S3, S3, _win(o2[:np_], 0, n_i, 0, kd, 1), Alu.add
                )
                r = kd
                while r > 2:
                    p2 = 1 << (r.bit_length() - 1)
                    h_ = r // 2 if p2 == r else r - p2
                    keep = r - h_
                    nc.vector.tensor_tensor(
                        S3[:, :, 0:h_], S3[:, :, 0:h_], S3[:, :, keep : keep + h_], Alu.min
                    )
                    r = keep
                if r == 2:
                    nc.vector.tensor_tensor(d2v, S3[:, :, 0], S3[:, :, 1], Alu.min)
                else:
                    nc.vector.tensor_copy(d2v, S3[:, :, 0])
                # k = 0 candidate
                nc.vector.tensor_tensor(
                    d2v, d2v, AP(gta.tensor, gta.offset + col0, [list(gta.ap[0]), [1, n_i]]),
                    Alu.min,
                )

            def far_matmul_into(ps_ap, j0, jn, icol0, n_i):
                """ps_ap[:, :] = sum_y E[y, j0:j0+jn] * W[y, icol0:+n_i]."""
                for yc, (h0, hs) in enumerate(CHUNKS):
                    nc.tensor.matmul(
                        ps_ap,
                        eslice(yc, j0, jn, hs),
                        wb[:hs, yc * W + icol0 : yc * W + icol0 + n_i],
                        start=(yc == 0),
                        stop=(yc == 2),
                    )

            def far_ln_act(ps, np_, n_i):
                lnm = d2p.tile([128, W], dt, tag="lnm")
                nc.scalar.activation(
                    lnm[:np_, :n_i], ps[:np_, :n_i], ActFn.Ln,
                    bias=wb[:np_, 3 * W : 3 * W + 1],
                )
                return lnm

            dbg_row = [0]

            def emit_out(d2v, np_, n_i, dsts, wide_exp=False):
                if DEBUG:
                    r0 = dbg_row[0]
                    nc.sync.dma_start(dbg_d[r0 : r0 + np_, 0:n_i], d2v)
                    dbg_row[0] += 128
                out_t = outp.tile([128, 3 * W], F32, tag="out")
                o3 = out_t[:np_, : 3 * n_i].rearrange("p (s i) -> p s i", s=3)
                if wide_exp and use_far:
                    # pre-scale d2 by 1/den_s on DVE (4x-mode tensor_scalar,
                    # ~72ns each), then ONE wide exp covers all three planes
                    # -- shortens the post-merge ACT chain on late blocks
                    qt = d2p.tile([128, 3 * W], dt, tag="qt")
                    q3 = qt[:np_, : 3 * n_i].rearrange("p (s i) -> p s i", s=3)
                    for si in range(3):
                        nc.vector.tensor_scalar(
                            q3[:, si, :], d2v, float(1.0 / dens[si]), 0.0,
                            Alu.mult, Alu.add,
                        )
                    nc.scalar.activation(
                        o3[:, :, :], q3, ActFn.Exp, scale=-1.0
                    )
                else:
                    for si in range(3):
                        nc.scalar.activation(
                            o3[:, si, :], d2v, ActFn.Exp, scale=float(-1.0 / dens[si])
                        )
                for psl, dst in dsts:
                    nc.sync.dma_start(dst, o3[psl])

            # --- far-field matmuls + Lns (PE/ACT; emitted first so the
            # in-order DVE queue is never blocked by a late Ln) ---
            lnms = [None, None, None]
            d2vs = [None, None, None]
            if use_far:
                ps0 = psmm.tile([128, W], F32, tag="mm0")
                far_matmul_into(ps0[:, :], 0, 128, 0, W)
                lnms[0] = far_ln_act(ps0, 128, W)
                ps1 = psmm.tile([128, W], F32, tag="mm1")
                far_matmul_into(ps1[:, :], 128, 128, 0, W)
                lnms[1] = far_ln_act(ps1, 128, W)
                lnm2 = d2p.tile([128, W], dt, tag="lnm", name="lnm2")
                for hi in range(2):
                    psh = psmm.tile([128, HH], F32, tag=f"mm2{hi}", name=f"mm2{hi}")
                    for yc, (h0, hs) in enumerate(CHUNKS):
                        nc.tensor.matmul(
                            psh[hi * 64 : hi * 64 + 64, :],
                            eslice(yc, 256, 64, hs),
                            wb[:hs, yc * W + hi * HH : yc * W + hi * HH + HH],
                            start=(yc == 0),
                            stop=(yc == 2),
                        )
                    nc.scalar.activation(
                        lnm2[hi * 64 : hi * 64 + 64, :HH],
                        psh[hi * 64 : hi * 64 + 64, :HH], ActFn.Ln,
                        bias=wb[hi * 64 : hi * 64 + 64, 3 * W : 3 * W + 1],
                    )
                lnms[2] = lnm2

            # --- near fields: jb0/jb1 on DVE, packed wc2 on Pool ---
            def finish(bi, np_, n_i, dsts, i0=0, wide_exp=False):
                d2v = d2vs[bi][:np_, i0 : i0 + n_i]
                if use_far:
                    lnv = lnms[bi][:np_, i0 : i0 + n_i]
                    d2f = d2p.tile([128, W], dt, tag="d2f")
                    nc.vector.tensor_scalar(
                        d2f[:np_, :n_i], lnv,
                        float(-1.0 / s), float(KF * KF), Alu.mult, Alu.add,
                    )
                    nc.vector.tensor_tensor(d2v, d2v, d2f[:np_, :n_i], Alu.min)
                emit_out(d2v, np_, n_i, dsts, wide_exp=wide_exp)

            dst_jb = [
                [(slice(0, 128), AP(y_d.tensor, jb * 128 * H, [[H, 128], [W * H, 3], [1, W]]))]
                for jb in range(2)
            ]
            dst_w2 = [
                (slice(0, 64), AP(y_d.tensor, 256 * H, [[H, 64], [W * H, 3], [1, HH]])),
                (slice(64, 128), AP(y_d.tensor, 256 * H + HH, [[H, 64], [W * H, 3], [1, HH]])),
            ]

            for bi in range(3):
                d2vs[bi] = d2p.tile([128, W], dt, tag="d2", name=f"d2t{bi}")

            def dst_jb_half(jb, i0, n_i=HH):
                return [(slice(0, 128),
                         AP(y_d.tensor, jb * 128 * H + i0, [[H, 128], [W * H, 3], [1, n_i]]))]

            # jb nears run per i-half so merges+exps trickle into ACT early;
            # wc2 (pk ready latest) slots mid-sequence (Pool has no TT-min,
            # so all nears live on DVE)
            near_dve(g2t[0][:128], RP, HH, d2vs[0][:, 0:HH], KF - 1)
            finish(0, 128, HH, dst_jb_half(0, 0), i0=0)
            near_dve(g2t[0][:128], RP + HH, HH, d2vs[0][:, HH:W], KF - 1)
            finish(0, 128, HH, dst_jb_half(0, HH), i0=HH)
            near_dve(pk[:128], RP, HH, d2vs[2][:, :HH], KF - 1)
            finish(2, 128, HH, dst_w2, wide_exp=True)
            near_dve(g2t[1][:128], RP, HH, d2vs[1][:, 0:HH], KF - 1)
            finish(1, 128, HH, dst_jb_half(1, 0), i0=0)
            near_dve(g2t[1][:128], RP + HH, HH, d2vs[1][:, HH:W], KF - 1)
            finish(1, 128, HH, dst_jb_half(1, HH), i0=HH, wide_exp=True)

    import concourse.bacc as _bacc_mod

    _orig_gat = _bacc_mod.get_activation_tables

    def _pin_act_tables(arch):
        t = _orig_gat(arch)
        return {
            k: (v if k == "natural_log_exp_and_others" else set())
            for k, v in t.items()
        }

    _bacc_mod.get_activation_tables = _pin_act_tables
    try:
        nc.compile()
    finally:
        _bacc_mod.get_activation_tables = _orig_gat
    return nc


def _host_prep(imgs):
    """Exact host-side analysis: max d2 over seeded images -> R (far-band
    width), s (far-field temperature), fp16 viability."""
    u = (np.float32(1.0) - imgs) * np.float32(127.5)
    m = u < np.float32(1.0)
    wi = np.arange(W, dtype=np.float32)
    last = np.maximum.accumulate(np.where(m, wi, np.float32(-BIG)), axis=2)
    nxt = np.minimum.accumulate(
        np.where(m, wi, np.float32(2 * BIG))[:, :, ::-1], axis=2
    )[:, :, ::-1]
    g = np.minimum(np.minimum(wi - last, nxt - wi), np.float32(BIG)).astype(np.float32)
    g2 = g * g
    seeded = m.any(axis=(1, 2))
    if not seeded.any():
        return 2, 0.1, True, 4.0
    D = g2.copy()
    o = 0
    while True:
        Mx = float(D[seeded].max())
        if o * o >= Mx or o >= H - 1:
            break
        o += 1
        c = np.float32(o * o)
        D[:, o:, :] = np.minimum(D[:, o:, :], g2[:, :-o, :] + c)
        D[:, :-o, :] = np.minimum(D[:, :-o, :], g2[:, o:, :] + c)
    maxd2 = float(D[seeded].max())
    R = max(KF_NEAR + 1, min(H - 1, int(math.ceil(math.sqrt(maxd2)))))
    s = 87.0 / (maxd2 - KF_NEAR * KF_NEAR + 30.0)
    fp16_ok = maxd2 <= 2047.0
    return R, float(np.float32(s)), fp16_ok, maxd2


def _consts(R, s, fp16):
    dt = np.float16 if fp16 else np.float32
    use_far = fp16
    KF = KF_NEAR if use_far else R + 1
    NO2 = KF - 1
    o2 = ((np.arange(NO2) + 1.0) ** 2)[None, :].repeat(128, 0)
    idt = np.eye(128)
    out = {"consts": np.concatenate([o2, idt], axis=1).astype(dt)}
    if use_far:
        wbm = np.zeros((128, 3 * W + 1), np.float32)
        wbm[:, 3 * W] = LN_EPS
        for c, (h0, hs) in enumerate(CHUNKS):
            y = (h0 + np.arange(hs))[:, None].astype(np.float64)
            i = np.arange(W)[None, :].astype(np.float64)
            dd = np.abs(y - i)
            band = (dd >= KF) & (dd <= R)
            wbm[:hs, c * W : (c + 1) * W] = np.where(
                band, np.exp(-s * ((y - i) ** 2 - KF * KF)), 0.0
            ).astype(np.float32)
        import ml_dtypes

        out["wband"] = wbm.astype(ml_dtypes.bfloat16)
    return out


def get_program(R, s, fp16):
    key = (R, round(s, 6), fp16)
    if key not in _prog_cache:
        _prog_cache[key] = _build(R, s, fp16)
    return _prog_cache[key]


def kernel(inputs):
    inputs = np.asarray(inputs, dtype=np.float32)
    Bn = inputs.shape[0]
    imgs = np.moveaxis(inputs, -1, 1).reshape(Bn * 2, H, W)
    assert imgs.shape[0] == NCORES, f"expected {NCORES} folded images, got {imgs.shape[0]}"

    R, s, fp16, _ = _host_prep(imgs)
    nc = get_program(R, s, fp16)
    cst = _consts(R, s, fp16)
    in_maps = [
        {"x": np.ascontiguousarray(imgs[i]), **cst} for i in range(NCORES)
    ]
    res = run_bass_kernel_spmd(nc, in_maps, list(range(NCORES)))
    out = np.empty((Bn, H, W, 6), np.float32)
    for core in range(NCORES):
        planes = res.results[core]["y"]  # [3, W, H]
        b, c = divmod(core, 2)
        for si in range(3):
            out[b, :, :, c * 3 + si] = planes[si].T
    return out


# revision 63
# speedup vs baseline: 1.0052x; 1.0052x over previous
"""Trainium2 Bass kernel for nn_Distance (exact EDT + Gaussian click maps).

Computes, for inputs [4, 320, 320, 2] f32 in [0,1):
  restored = uint8((1-x)*127.5); zero-mask = (restored == 0)
  d2 = squared Euclidean distance transform of the zero-mask
       (per image, channel folded into batch -> 8 independent images)
  out[..., c*3+s] = exp(-d2_c / (2*sigma_s^2)), sigmas = [0.02,0.08,0.16]*320

Sharding: pure data parallel, one folded image (b, c) per NeuronCore (8 cores).

Device algorithm (decomposed transposed relative to the reference):
  phase A: per-row 1D distances along W via two fused distance-recurrence
           scans on DVE (state = nm*state + nm gives 0 at seeds, +1 runs);
           the zero-mask is a single threshold compare x < T (T found by
           exact fp32 bisection; provably equal to the uint8-trunc mask).
  phase B: d2[j,i] = min_{|k|<=R} g2t[j,i+k] + k^2, split into
    - near band |k| < KF=5: exact windowed pair-min + k^2-add + log-fold
      on DVE over PE-transposed fp16 row-distance squares, processed per
      160-col i-half so merges/exps/stores pipeline into ACT/SP early
      (Pool has no tensor-min; it runs masks, E-squares and the wc2
      repack instead);
    - far band KF <= |k| <= R: softmin on the TensorEngine:
      M[j,i] = sum_y exp(-s*g2[y,j]) * exp(-s*((y-i)^2-KF^2)) accumulated
      over y-chunks in PSUM (bf16 matmuls vs a constant banded matrix;
      bf16 keeps fp32's exponent range), then d2far = KF^2 - ln(M)/s
      (ACT Ln with an SBUF eps bias + DVE affine). s is chosen on the
      host so every winner-relevant term stays inside fp32 range; the
      softmin bias only touches pixels whose winner is >= KF away, where
      all three Gaussians are flat (measured end-to-end rel err ~2e-3 on
      device vs the 2e-2 gate). The activation-function table is pinned
      to natural_log_exp_and_others (square+exp+ln+copy, 400-bucket
      tables) so only one LoadActFuncSet is ever issued.
  merge + 3 exps per half-block on ACT, fused 3-plane output stores.
R and s derive from a host-side exact EDT of the actual input. The near
band is exact in fp16 (winning candidates are integers <= 2047; padded or
overflowed losers saturate to +inf and never win).
"""

import math
import os
import sys

import numpy as np

for _p in ("/opt/trn_rl_repo", "/root/.axon_site/_ro/trn_rl_repo"):
    if os.path.isdir(_p) and _p not in sys.path:
        sys.path.insert(0, _p)

import concourse.bass as bass  # noqa: E402
import concourse.tile as tile  # noqa: E402
from concourse import bacc, mybir  # noqa: E402
from concourse.ap import AP  # noqa: E402
from concourse.bass_utils import run_bass_kernel_spmd  # noqa: E402

H = 320
W = 320
NCORES = 8
BIG = 1e5
LENGTH = 320
# exact fp32 threshold: (uint8-trunc zero mask) == (x >= T_ZERO); bisected
# over fp32 and verified exhaustively around the boundary.
T_ZERO = float(np.float32(0.99215686))
C0 = 1024.0  # scan init: "no seed yet" distance offset (< 2048 for fp16)
KF_NEAR = 5  # near band: exact k in [0, KF); far band via PE softmin
DEBUG = False  # adds a dbg output with pre-exp d2 per block
LN_EPS = 1e-37  # Ln bias floor: maps empty far fields to d2 > maxd2

F32 = mybir.dt.float32
F16 = mybir.dt.float16
BF16 = mybir.dt.bfloat16
Alu = mybir.AluOpType
ActFn = mybir.ActivationFunctionType

CHUNKS = [(0, 128), (128, 128), (256, 64)]

_prog_cache: dict = {}


def _denoms():
    sig = (np.float32(np.array([0.02, 0.08, 0.16], np.float32)) * np.float32(LENGTH)).astype(np.float32)
    return (np.float32(2.0) * sig * sig).astype(np.float32)


def _win(apo, col0, ni, istep, nk, kstep):
    """3D overlapping-window view of a 2D [P, F] AP: [p, i, k] -> col0 + i*istep + k*kstep."""
    return AP(apo.tensor, apo.offset + col0, [list(apo.ap[0]), [istep, ni], [kstep, nk]])


def _build(R, s, fp16):
    """Build + compile the per-core program. Returns the Bacc module."""
    dt = F16 if fp16 else F32
    PADV = 60000.0 if fp16 else 1e20
    use_far = fp16
    KF = KF_NEAR if use_far else R + 1
    KD = KF - 1  # DVE near k in [1, 1+KD); k=0 merged separately
    assert KD >= 2
    RP = 16 if use_far else ((R + 15) // 16) * 16
    PADH = RP + H + RP
    HH = H // 2
    pack_wc2 = True
    dens = _denoms()

    nc = bacc.Bacc("TRN2", target_bir_lowering=False, debug=False, num_devices=NCORES)
    x_d = nc.dram_tensor("x", [H, W], F32, kind="ExternalInput").ap()
    NO2 = KF - 1  # o2 table: k^2 for k = 1..KF-1
    cst_d = nc.dram_tensor("consts", [128, NO2 + 128], dt, kind="ExternalInput").ap()
    wb_d = None
    if use_far:
        wb_d = nc.dram_tensor("wband", [128, 3 * W + 1], BF16, kind="ExternalInput").ap()
    y_d = nc.dram_tensor("y", [3, W, H], F32, kind="ExternalOutput").ap()
    dbg_d = None
    if DEBUG:
        dbg_d = nc.dram_tensor("dbg", [3 * 128, W], F16, kind="ExternalOutput").ap()

    with tile.TileContext(nc) as tc:
        with (
            tc.tile_pool(name="const", bufs=1) as constp,
            tc.tile_pool(name="xp", bufs=1) as xp,
            tc.tile_pool(name="pa", bufs=3) as pa,
            tc.tile_pool(name="ep", bufs=1) as ep,
            tc.tile_pool(name="g2tp", bufs=4) as g2tp,
            tc.tile_pool(name="sbig", bufs=2) as sbig,
            tc.tile_pool(name="accp", bufs=2) as accp,
            tc.tile_pool(name="d2p", bufs=3) as d2p,
            tc.tile_pool(name="outp", bufs=3) as outp,
            tc.tile_pool(name="psum", bufs=4, space="PSUM") as psump,
            tc.tile_pool(name="psmm", bufs=1, space="PSUM") as psmm,
        ):
            # ---- input DMAs (x spread over SP and ACT queues) ----
            x0 = xp.tile([128, W], F32, tag="x0")
            nc.sync.dma_start(x0[:], x_d[0:128, :])
            x1 = xp.tile([128, W], F32, tag="x1")
            nc.scalar.dma_start(x1[:], x_d[128:256, :])
            x2 = xp.tile([128, W], F32, tag="x2")
            nc.sync.dma_start(x2[:64], x_d[256:320, :])
            cst = constp.tile([128, NO2 + 128], dt)
            nc.scalar.dma_start(cst[:], cst_d)
            o2 = cst[:, 0:NO2]
            idt = cst[:, NO2 : NO2 + 128]
            wb = None
            if use_far:
                wb = constp.tile([128, 3 * W + 1], BF16, tag="wb")
                nc.scalar.dma_start(wb[:], wb_d)
            xtiles = [x0, x1, x2]

            # transposed g2 tiles (pads memset by DVE during the DMA wait)
            g2t = []
            for wc in range(3):
                t = g2tp.tile([128, PADH], dt, tag="g2t")
                nc.vector.memset(t[:, 0:RP], PADV)
                nc.vector.memset(t[:, RP + H : PADH], PADV)
                g2t.append(t)
            pk = None
            if pack_wc2:
                pk = g2tp.tile([128, RP + HH + RP], dt, tag="pk")

            # ---- phase A ----
            def phase_a_mask(hc, eng):
                h0, hs = CHUNKS[hc]
                nm = pa.tile([128, W], dt, tag="nm")
                eng.tensor_scalar(
                    nm[:hs], xtiles[hc][:hs], 1.0, T_ZERO, Alu.mult, Alu.is_lt
                )
                return nm

            def phase_a_scan(hc, nm):
                """g = min(left-dist, right-dist); no clamp -- squares of
                no-seed runs overflow fp16 to +inf, which loses every min."""
                h0, hs = CHUNKS[hc]
                dl = pa.tile([128, W], dt, tag="dl")
                nc.vector.tensor_tensor_scan(
                    dl[:hs], nm[:hs], nm[:hs], C0, Alu.mult, Alu.add
                )
                dr = pa.tile([128, W], dt, tag="dr")
                nc.vector.tensor_tensor_scan(
                    dr[:hs, ::-1], nm[:hs, ::-1], nm[:hs, ::-1], C0, Alu.mult, Alu.add
                )
                g_t = pa.tile([128, W], dt, tag="g")
                nc.vector.tensor_tensor(g_t[:hs], dl[:hs], dr[:hs], Alu.min)
                return g_t

            g2pre01 = None
            E01 = None
            E2 = None
            if use_far:
                g2pre01 = ep.tile([128, 2 * W], F32, tag="g2pre01")
                E01 = ep.tile([128, 2 * W], BF16, tag="E01")
                E2 = ep.tile([128, W], BF16, tag="E2")

            def eslice(yc, j0, jn, hs):
                if yc < 2:
                    return E01[:hs, yc * W + j0 : yc * W + j0 + jn]
                return E2[:hs, j0 : j0 + jn]

            def phase_a_post(hc, g_t):
                """PE-transpose g into PSUM, square during the ACT copy-out;
                also build the far-field factor E = exp(-s * g^2)."""
                h0, hs = CHUNKS[hc]
                for wc, (w0, ws) in enumerate(CHUNKS):
                    pt = psump.tile([128, 128], dt, tag="pt")
                    nc.tensor.transpose(
                        pt[:ws, :hs], g_t[:hs, w0 : w0 + ws], idt[:hs, :hs]
                    )
                    nc.scalar.activation(
                        g2t[wc][:ws, RP + h0 : RP + h0 + hs], pt[:ws, :hs], ActFn.Square
                    )
                if use_far:
                    if hc < 2:
                        nc.gpsimd.tensor_tensor(
                            g2pre01[:hs, hc * W : (hc + 1) * W], g_t[:hs], g_t[:hs],
                            Alu.mult,
                        )
                        if hc == 1:
                            nc.scalar.activation(
                                E01[:], g2pre01[:], ActFn.Exp, scale=float(-s)
                            )
                    else:
                        g2pre = pa.tile([128, W], F32, tag="g2pre")
                        nc.gpsimd.tensor_tensor(g2pre[:hs], g_t[:hs], g_t[:hs], Alu.mult)
                        nc.scalar.activation(
                            E2[:hs], g2pre[:hs], ActFn.Exp, scale=float(-s)
                        )

            nm0 = phase_a_mask(0, nc.vector)
            nm1 = phase_a_mask(1, nc.gpsimd)
            nm2 = phase_a_mask(2, nc.gpsimd)
            g0 = phase_a_scan(0, nm0)
            phase_a_post(0, g0)
            g1 = phase_a_scan(1, nm1)
            phase_a_post(1, g1)
            g2_ = phase_a_scan(2, nm2)
            phase_a_post(2, g2_)

            if pack_wc2:
                nc.gpsimd.tensor_copy(pk[0:64, :], g2t[2][0:64, 0 : 2 * RP + HH])
                nc.gpsimd.tensor_copy(pk[64:128, :], g2t[2][0:64, HH : 2 * RP + 2 * HH])

            # ---- phase B ----
            def near_dve(gta, col0, n_i, d2v, kd=KD):
                """d2v = min_{k in [0, 1+kd)} min(g[i+k], g[i-k]) + k^2."""
                np_ = gta.shape[0]
                S = sbig.tile([128, W * KD], dt, tag="Sd")
                assert n_i * kd <= W * KD
                S3 = AP(
                    S.tensor, S.offset,
                    [list(S[:np_].ap[0]), [kd, n_i], [1, kd]],
                )
                nc.vector.tensor_tensor(
                    S3, _win(gta, col0 + 1, n_i, 1, kd, 1),
                    _win(gta, col0 - 1, n_i, 1, kd, -1), Alu.min,
                )
                nc.vector.tensor_tensor(
                    S3, S3, _win(o2[:np_], 0, n_i, 0, kd, 1), Alu.add
                )
                r = kd
                while r > 2:
                    p2 = 1 << (r.bit_length() - 1)
                    h_ = r // 2 if p2 == r else r - p2
                    keep = r - h_
                    nc.vector.tensor_tensor(
                        S3[:, :, 0:h_], S3[:, :, 0:h_], S3[:, :, keep : keep + h_], Alu.min
                    )
                    r = keep
                if r == 2:
                    nc.vector.tensor_tensor(d2v, S3[:, :, 0], S3[:, :, 1], Alu.min)
                else:
                    nc.vector.tensor_copy(d2v, S3[:, :, 0])
                # k = 0 candidate
                nc.vector.tensor_tensor(
                    d2v, d2v, AP(gta.tensor, gta.offset + col0, [list(gta.ap[0]), [1, n_i]]),
                    Alu.min,
                )

            def far_matmul_into(ps_ap, j0, jn, icol0, n_i):
                """ps_ap[:, :] = sum_y E[y, j0:j0+jn] * W[y, icol0:+n_i]."""
                for yc, (h0, hs) in enumerate(CHUNKS):
                    nc.tensor.matmul(
                        ps_ap,
                        eslice(yc, j0, jn, hs),
                        wb[:hs, yc * W + icol0 : yc * W + icol0 + n_i],
                        start=(yc == 0),
                        stop=(yc == 2),
                    )

            def far_ln_act(ps, np_, n_i):
                lnm = d2p.tile([128, W], dt, tag="lnm")
                nc.scalar.activation(
                    lnm[:np_, :n_i], ps[:np_, :n_i], ActFn.Ln,
                    bias=wb[:np_, 3 * W : 3 * W + 1],
                )
                return lnm

            dbg_row = [0]

            def emit_out(d2v, np_, n_i, dsts, wide_exp=False):
                if DEBUG:
                    r0 = dbg_row[0]
                    nc.sync.dma_start(dbg_d[r0 : r0 + np_, 0:n_i], d2v)
                    dbg_row[0] += 128
                out_t = outp.tile([128, 3 * W], F32, tag="out")
                o3 = out_t[:np_, : 3 * n_i].rearrange("p (s i) -> p s i", s=3)
                if wide_exp and use_far:
                    # pre-scale d2 by 1/den_s on DVE (4x-mode tensor_scalar,
                    # ~72ns each), then ONE wide exp covers all three planes
                    # -- shortens the post-merge ACT chain on late blocks
                    qt = d2p.tile([128, 3 * W], dt, tag="qt")
                    q3 = qt[:np_, : 3 * n_i].rearrange("p (s i) -> p s i", s=3)
                    for si in range(3):
                        nc.vector.tensor_scalar(
                            q3[:, si, :], d2v, float(1.0 / dens[si]), 0.0,
                            Alu.mult, Alu.add,
                        )
                    nc.scalar.activation(
                        o3[:, :, :], q3, ActFn.Exp, scale=-1.0
                    )
                else:
                    for si in range(3):
                        nc.scalar.activation(
                            o3[:, si, :], d2v, ActFn.Exp, scale=float(-1.0 / dens[si])
                        )
                for psl, dst in dsts:
                    nc.sync.dma_start(dst, o3[psl])

            # --- far-field matmuls + Lns (PE/ACT; emitted first so the
            # in-order DVE queue is never blocked by a late Ln) ---
            lnms = [None, None, None]
            d2vs = [None, None, None]
            if use_far:
                ps0 = psmm.tile([128, W], F32, tag="mm0")
                far_matmul_into(ps0[:, :], 0, 128, 0, W)
                lnms[0] = far_ln_act(ps0, 128, W)
                ps1 = psmm.tile([128, W], F32, tag="mm1")
                far_matmul_into(ps1[:, :], 128, 128, 0, W)
                lnms[1] = far_ln_act(ps1, 128, W)
                lnm2 = d2p.tile([128, W], dt, tag="lnm", name="lnm2")
                for hi in range(2):
                    psh = psmm.tile([128, HH], F32, tag=f"mm2{hi}", name=f"mm2{hi}")
                    for yc, (h0, hs) in enumerate(CHUNKS):
                        nc.tensor.matmul(
                            psh[hi * 64 : hi * 64 + 64, :],
                            eslice(yc, 256, 64, hs),
                            wb[:hs, yc * W + hi * HH : yc * W + hi * HH + HH],
                            start=(yc == 0),
                            stop=(yc == 2),
                        )
                    nc.scalar.activation(
                        lnm2[hi * 64 : hi * 64 + 64, :HH],
                        psh[hi * 64 : hi * 64 + 64, :HH], ActFn.Ln,
                        bias=wb[hi * 64 : hi * 64 + 64, 3 * W : 3 * W + 1],
                    )
                lnms[2] = lnm2

            # --- near fields: jb0/jb1 on DVE, packed wc2 on Pool ---
            def finish(bi, np_, n_i, dsts, i0=0, wide_exp=False):
                d2v = d2vs[bi][:np_, i0 : i0 + n_i]
                if use_far:
                    # the affine runs on Pool: its Ln input is ready early
                    # (~11us) and off the near-band chain, so this frees DVE
                    # mid-stream at zero dependency cost
                    lnv = lnms[bi][:np_, i0 : i0 + n_i]
                    d2f = d2p.tile([128, W], dt, tag="d2f")
                    nc.gpsimd.tensor_scalar(
                        d2f[:np_, :n_i], lnv,
                        float(-1.0 / s), float(KF * KF), Alu.mult, Alu.add,
                    )
                    nc.vector.tensor_tensor(d2v, d2v, d2f[:np_, :n_i], Alu.min)
                emit_out(d2v, np_, n_i, dsts, wide_exp=wide_exp)

            dst_jb = [
                [(slice(0, 128), AP(y_d.tensor, jb * 128 * H, [[H, 128], [W * H, 3], [1, W]]))]
                for jb in range(2)
            ]
            dst_w2 = [
                (slice(0, 64), AP(y_d.tensor, 256 * H, [[H, 64], [W * H, 3], [1, HH]])),
                (slice(64, 128), AP(y_d.tensor, 256 * H + HH, [[H, 64], [W * H, 3], [1, HH]])),
            ]

            for bi in range(3):
                d2vs[bi] = d2p.tile([128, W], dt, tag="d2", name=f"d2t{bi}")

            def dst_jb_half(jb, i0, n_i=HH):
                return [(slice(0, 128),
                         AP(y_d.tensor, jb * 128 * H + i0, [[H, 128], [W * H, 3], [1, n_i]]))]

            # jb nears run per i-half so merges+exps trickle into ACT early;
            # wc2 (pk ready latest) slots mid-sequence (Pool has no TT-min,
            # so all nears live on DVE)
            near_dve(g2t[0][:128], RP, HH, d2vs[0][:, 0:HH], KF - 1)
            finish(0, 128, HH, dst_jb_half(0, 0), i0=0)
            near_dve(g2t[0][:128], RP + HH, HH, d2vs[0][:, HH:W], KF - 1)
            finish(0, 128, HH, dst_jb_half(0, HH), i0=HH)
            near_dve(pk[:128], RP, HH, d2vs[2][:, :HH], KF - 1)
            finish(2, 128, HH, dst_w2, wide_exp=True)
            near_dve(g2t[1][:128], RP, HH, d2vs[1][:, 0:HH], KF - 1)
            finish(1, 128, HH, dst_jb_half(1, 0), i0=0)
            near_dve(g2t[1][:128], RP + HH, HH, d2vs[1][:, HH:W], KF - 1)
            finish(1, 128, HH, dst_jb_half(1, HH), i0=HH, wide_exp=True)

    import concourse.bacc as _bacc_mod

    _orig_gat = _bacc_mod.get_activation_tables

    def _pin_act_tables(arch):
        t = _orig_gat(arch)
        return {
            k: (v if k == "natural_log_exp_and_others" else set())
            for k, v in t.items()
        }

    _bacc_mod.get_activation_tables = _pin_act_tables
    try:
        nc.compile()
    finally:
        _bacc_mod.get_activation_tables = _orig_gat
    return nc


def _host_prep(imgs):
    """Exact host-side analysis: max d2 over seeded images -> R (far-band
    width), s (far-field temperature), fp16 viability."""
    u = (np.float32(1.0) - imgs) * np.float32(127.5)
    m = u < np.float32(1.0)
    wi = np.arange(W, dtype=np.float32)
    last = np.maximum.accumulate(np.where(m, wi, np.float32(-BIG)), axis=2)
    nxt = np.minimum.accumulate(
        np.where(m, wi, np.float32(2 * BIG))[:, :, ::-1], axis=2
    )[:, :, ::-1]
    g = np.minimum(np.minimum(wi - last, nxt - wi), np.float32(BIG)).astype(np.float32)
    g2 = g * g
    seeded = m.any(axis=(1, 2))
    if not seeded.any():
        return 2, 0.1, True, 4.0
    D = g2.copy()
    o = 0
    while True:
        Mx = float(D[seeded].max())
        if o * o >= Mx or o >= H - 1:
            break
        o += 1
        c = np.float32(o * o)
        D[:, o:, :] = np.minimum(D[:, o:, :], g2[:, :-o, :] + c)
        D[:, :-o, :] = np.minimum(D[:, :-o, :], g2[:, o:, :] + c)
    maxd2 = float(D[seeded].max())
    R = max(KF_NEAR + 1, min(H - 1, int(math.ceil(math.sqrt(maxd2)))))
    s = 87.0 / (maxd2 - KF_NEAR * KF_NEAR + 30.0)
    fp16_ok = maxd2 <= 2047.0
    return R, float(np.float32(s)), fp16_ok, maxd2


def _consts(R, s, fp16):
    dt = np.float16 if fp16 else np.float32
    use_far = fp16
    KF = KF_NEAR if use_far else R + 1
    NO2 = KF - 1
    o2 = ((np.arange(NO2) + 1.0) ** 2)[None, :].repeat(128, 0)
    idt = np.eye(128)
    out = {"consts": np.concatenate([o2, idt], axis=1).astype(dt)}
    if use_far:
        wbm = np.zeros((128, 3 * W + 1), np.float32)
        wbm[:, 3 * W] = LN_EPS
        for c, (h0, hs) in enumerate(CHUNKS):
            y = (h0 + np.arange(hs))[:, None].astype(np.float64)
            i = np.arange(W)[None, :].astype(np.float64)
            dd = np.abs(y - i)
            band = (dd >= KF) & (dd <= R)
            wbm[:hs, c * W : (c + 1) * W] = np.where(
                band, np.exp(-s * ((y - i) ** 2 - KF * KF)), 0.0
            ).astype(np.float32)
        import ml_dtypes

        out["wband"] = wbm.astype(ml_dtypes.bfloat16)
    return out


def get_program(R, s, fp16):
    key = (R, round(s, 6), fp16)
    if key not in _prog_cache:
        _prog_cache[key] = _build(R, s, fp16)
    return _prog_cache[key]


def kernel(inputs):
    inputs = np.asarray(inputs, dtype=np.float32)
    Bn = inputs.shape[0]
    imgs = np.moveaxis(inputs, -1, 1).reshape(Bn * 2, H, W)
    assert imgs.shape[0] == NCORES, f"expected {NCORES} folded images, got {imgs.shape[0]}"

    R, s, fp16, _ = _host_prep(imgs)
    nc = get_program(R, s, fp16)
    cst = _consts(R, s, fp16)
    in_maps = [
        {"x": np.ascontiguousarray(imgs[i]), **cst} for i in range(NCORES)
    ]
    res = run_bass_kernel_spmd(nc, in_maps, list(range(NCORES)))
    out = np.empty((Bn, H, W, 6), np.float32)
    for core in range(NCORES):
        planes = res.results[core]["y"]  # [3, W, H]
        b, c = divmod(core, 2)
        for si in range(3):
            out[b, :, :, c * 3 + si] = planes[si].T
    return out
